# revision 10
# baseline (speedup 1.0000x reference)
"""GPTQMarlinFP8Linear kernel for 8 Trainium2 NeuronCores.

C[32, 12288] = A[32,4096] @ dequant(fp8_quant(W))[12288,4096].T + bias

Strategy (column-parallel / tensor-parallel over out_features):
  - Host: quantize W to fp8-e4m3fn exactly as the reference does (scalar
    scale = 448/amax).  The quantized values are stored HALVED and encoded
    as TRN fp8_e4m3 (IEEE variant, max 240) so the PE can consume 1-byte
    weights directly; the x2 is folded into the A-side scaling.
  - Host: A_scaled = A * (recip/kappa) in fp32, split hi/lo into bf16 so the
    fp32 precision of A survives a bf16/fp8 matmul.  Stationary = [128, 64]
    (columns 0:32 hi, 32:64 lo).
  - Device (per core c): PSUM[64, 1536] += A_sT[k].T @ q'[k] over 32 k-tiles,
    plus one tiny K=2 matmul adding bias (hi/lo rows) to partitions 0:32.
    Epilogue: out[32,1536] = PSUM[0:32] + PSUM[32:64]  (single DVE add).
  - Gather: concatenate 8 core outputs along out_features.
"""

import os
import sys

for _p in ("/root/.axon_site", "/root/.axon_site/_ro/trn_rl_repo",
           "/root/.axon_site/_ro/pypackages", "/opt/trn_rl_repo", "/opt/pypackages"):
    if os.path.isdir(_p) and _p not in sys.path:
        sys.path.append(_p)

import numpy as np
import ml_dtypes

import concourse.bass as bass
import concourse.bacc as bacc
import concourse.mybir as mybir
import concourse.tile as tile
from concourse.bass_utils import run_bass_kernel_spmd

N_CORES = 8
M = 32            # rows of A
K = 4096          # in_features
NOUT = 12288      # out_features
NSH = NOUT // N_CORES   # 1536 out_features per core
P = 128           # partitions / k-tile size
KT = K // P       # 32 k-tiles
M2 = 2 * M        # hi|lo stationary width
NB = 512          # psum bank chunk (fp32)
NCHUNKS = NSH // NB     # 3
FP8_MAX = np.float32(448.0)

# q storage dtype: 'fp8' (TRN e4m3, values halved) or 'bf16'
QMODE = os.environ.get("QK_QMODE", "fp8")
DMA_CHUNK = int(os.environ.get("QK_DMA_CHUNK", "4"))   # k-tiles per weight DMA
WBUFS = int(os.environ.get("QK_WBUFS", "8"))

_BF16 = mybir.dt.bfloat16
_F32 = mybir.dt.float32
_QDT = {"fp8": mybir.dt.float8e4, "bf16": _BF16}[QMODE]
_QNP = {"fp8": ml_dtypes.float8_e4m3, "bf16": ml_dtypes.bfloat16}[QMODE]
_KAPPA = {"fp8": np.float32(0.5), "bf16": np.float32(1.0)}[QMODE]

_cached_nc = None


def _build_nc():
    nc = bass.Bass()
    q_d = nc.declare_dram_parameter("q_sb", [P, KT, NSH], _QDT, isOutput=False)
    a_d = nc.declare_dram_parameter("a_sb", [P, KT, M2], _BF16, isOutput=False)
    bc_d = nc.declare_dram_parameter("bcoef", [2, M2], _BF16, isOutput=False)
    br_d = nc.declare_dram_parameter("brows", [2, NSH], _BF16, isOutput=False)
    o_d = nc.declare_dram_parameter("out", [M, NSH], _F32, isOutput=True)

    with tile.TileContext(nc) as tc:
        with (
            tc.tile_pool(name="wpool", bufs=WBUFS) as wpool,
            tc.tile_pool(name="apool", bufs=1) as apool,
            tc.tile_pool(name="opool", bufs=1) as opool,
            tc.tile_pool(name="psum", bufs=1, space="PSUM") as pspool,
        ):
            a_t = apool.tile([P, KT, M2], _BF16)
            nc.sync.dma_start(a_t[:], a_d[:])
            bc_t = apool.tile([2, M2], _BF16)
            nc.sync.dma_start(bc_t[:], bc_d[:])
            br_t = apool.tile([2, NSH], _BF16)
            nc.sync.dma_start(br_t[:], br_d[:])

            ps = pspool.tile([M2, NSH], _F32)

            for j in range(KT // DMA_CHUNK):
                w_t = wpool.tile([P, DMA_CHUNK, NSH], _QDT, tag="w")
                nc.sync.dma_start(
                    w_t[:], q_d[:, j * DMA_CHUNK:(j + 1) * DMA_CHUNK, :]
                )
                for kl in range(DMA_CHUNK):
                    k = j * DMA_CHUNK + kl
                    for n in range(NCHUNKS):
                        nc.tensor.matmul(
                            ps[:, n * NB:(n + 1) * NB],
                            a_t[:, k, :],
                            w_t[:, kl, n * NB:(n + 1) * NB],
                            start=(k == 0),
                            stop=False,
                        )
            # bias: out[m, n] += sum_k bcoef[k, m] * brows[k, n]; bcoef is 1
            # for m<32 so bias lands only on the hi half.
            for n in range(NCHUNKS):
                nc.tensor.matmul(
                    ps[:, n * NB:(n + 1) * NB],
                    bc_t[:],
                    br_t[:, n * NB:(n + 1) * NB],
                    start=False,
                    stop=True,
                )

            o_hi = opool.tile([M, NSH], _F32)
            nc.vector.tensor_copy(o_hi[:], ps[0:M, :])
            o_t = opool.tile([M, NSH], _F32)
            nc.vector.tensor_tensor(
                o_t[:], o_hi[:], ps[M:M2, :], mybir.AluOpType.add
            )
            nc.gpsimd.dma_start(o_d[:], o_t[:])
    # walrus in this environment allows at most 1 sync-wait per regular
    # instruction; split multi-waits into event-semaphore cascades.
    import bass_rust
    bass_rust.generate_event_semaphores(nc)
    return nc


def _get_nc():
    global _cached_nc
    if _cached_nc is None:
        _cached_nc = _build_nc()
    return _cached_nc


def _prep_inputs(A, weight, bias):
    A = np.ascontiguousarray(np.asarray(A, dtype=np.float32))
    W = np.ascontiguousarray(np.asarray(weight, dtype=np.float32))
    b = np.ascontiguousarray(np.asarray(bias, dtype=np.float32))

    amax = np.maximum(np.abs(W).max(), np.float32(1e-12)).astype(np.float32)
    scale = (FP8_MAX / amax).astype(np.float32)
    recip = (np.float32(1.0) / scale).astype(np.float32)

    q = np.clip(W * scale, -FP8_MAX, FP8_MAX).astype(ml_dtypes.float8_e4m3fn)
    # stored weights: q * kappa in the device dtype (exact for bf16; for fp8
    # the TRN e4m3 grid loses only a handful of subnormal LSBs ~1e-7 relative)
    qs = (q.astype(np.float32) * _KAPPA).astype(_QNP)

    # A side: fold recip/kappa into A, then split hi/lo bf16
    As = (A * (recip / _KAPPA)).astype(np.float32)
    As_hi = As.astype(ml_dtypes.bfloat16)
    As_lo = (As - As_hi.astype(np.float32)).astype(ml_dtypes.bfloat16)
    # a_sb[p, k, m] = As_{hi|lo}[m, k*128+p]
    a_sb = np.empty((P, KT, M2), dtype=ml_dtypes.bfloat16)
    a_sb[:, :, :M] = As_hi.T.reshape(KT, P, M).transpose(1, 0, 2)
    a_sb[:, :, M:] = As_lo.T.reshape(KT, P, M).transpose(1, 0, 2)

    bcoef = np.zeros((2, M2), dtype=ml_dtypes.bfloat16)
    bcoef[:, :M] = ml_dtypes.bfloat16(1.0)

    b_hi = b.astype(ml_dtypes.bfloat16)
    b_lo = (b - b_hi.astype(np.float32)).astype(ml_dtypes.bfloat16)

    in_maps = []
    for c in range(N_CORES):
        qc = qs[c * NSH:(c + 1) * NSH, :]            # [1536, 4096]
        # q_sb[p, k, n] = q[c*NSH+n, k*128+p]
        q_sb = np.ascontiguousarray(
            qc.T.reshape(KT, P, NSH).transpose(1, 0, 2)
        )
        brows = np.empty((2, NSH), dtype=ml_dtypes.bfloat16)
        brows[0] = b_hi[c * NSH:(c + 1) * NSH]
        brows[1] = b_lo[c * NSH:(c + 1) * NSH]
        in_maps.append(
            {"q_sb": q_sb, "a_sb": a_sb, "bcoef": bcoef, "brows": brows}
        )
    return in_maps


def kernel(A, weight, bias):
    nc = _get_nc()
    in_maps = _prep_inputs(A, weight, bias)
    trace = bool(int(os.environ.get("QK_TRACE", "0")))
    res = run_bass_kernel_spmd(
        nc, in_maps, core_ids=list(range(N_CORES)), trace=trace
    )
    if trace and res.exec_time_ns is not None:
        print(f"HW exec time: {res.exec_time_ns} ns")
        kernel.last_exec_time_ns = res.exec_time_ns
        kernel.last_results = res
    out = np.concatenate(
        [res.results[c]["out"] for c in range(N_CORES)], axis=1
    )
    return np.ascontiguousarray(out.astype(np.float32))
